# revision 10
# baseline (speedup 1.0000x reference)
"""Trainium2 Bass kernel for CAttention:
    k      = einsum('bcit,i->bct', x, alpha)
    scores = einsum('bct,ts,bds->bcd', k, Wc, k)
    att    = softmax(scores, axis=-1)
    out    = einsum('bci,bint->bcnt', att, x)

Sharding: data-parallel over batch B=64 across 8 NeuronCores (8 batches/core).

Per-core layout (per batch b):
    X SBUF tile [128, 8192]: partition p = j*8 + d  (j in [0,16) = n-chunk,
    d in [0,8) = channel), free q = n2*64 + t with n = j*128 + n2.

    k-path : s[(j,d),t] = sum_n2 alpha[j*128+n2] * X  (DVE mul + strided reduce)
             kT[t,d]    = sum_(j,d') s * sel          (PE, s_t as stationary)
    scores : V = Wc @ kT (PE, WcT const); scores = kT.T @ V (PE)
    softmax: unnormalized exp on ACT (accum row-sum); 1/sum replicated via PE;
             normalization folded into the PSUM-evacuation scale.
    mix    : block-diag(e^T) [128,128] stationary, fp32r matmuls (1 cyc/row)
    out    : ACT evacuates PSUM -> SBUF bf16 with per-partition 1/sum scale,
             DMA out in bf16 (host upcasts); halves the write traffic.

DMA ring balance: each batch's input is split into two partition-halves, one
on the SP (sync) HWDGE ring and one on the ACT (scalar) ring; each batch's
output (bf16, staged in halves) likewise alternates rings. Both rings carry
~3.2MB/batch so the 16 SDMA engines always have two queues to round-robin
(measured: one loaded ring sustains ~210 GB/s, two loaded rings ~300+).
Emission is software-pipelined one batch ahead (a0 a1 b0 a2 b1 ... b7) so
input prefetch descriptors are never queued behind output DMAs on a ring.
Constants ride the gpsimd SWDGE queue to keep the HWDGE rings clean.
"""

import sys

for _p in ("/opt/trn_rl_repo",):
    if _p not in sys.path:
        sys.path.insert(0, _p)

import numpy as np

B, C, N, T = 64, 8, 2048, 64
NCORES = 8
BS = B // NCORES          # batches per core
J = 16                    # n-chunks on partitions
N2 = N // J               # 128, n-extent in free dim
P = J * C                 # 128 partitions
F = N2 * T                # 8192 free elems
QW = 512                  # mix matmul free width (one PSUM bank)
HF = F // 2               # output staging half

_PROGRAM_CACHE = {}


def _build_program():
    from contextlib import ExitStack

    import concourse.bacc as bacc
    from concourse import mybir, tile

    fp32 = mybir.dt.float32
    f32r = mybir.dt.float32r
    bf16 = mybir.dt.bfloat16
    nc = bacc.Bacc("TRN2", target_bir_lowering=False, debug=False)

    xs = nc.dram_tensor("xs", [BS, C, N, T], fp32, kind="ExternalInput").ap()
    ac = nc.dram_tensor("ac", [P, N2], fp32, kind="ExternalInput").ap()
    # packed: sel[0:8] | wcT[8:72] (rows 0-63) | id8[72:80] (rows 0-7) |
    #         rep[80:208] (rows 0-7) | mask[208:336]
    aux = nc.dram_tensor("aux", [P, 336], fp32, kind="ExternalInput").ap()
    out = nc.dram_tensor("out", [BS, C, N, T], bf16, kind="ExternalOutput").ap()

    Exp = mybir.ActivationFunctionType.Exp
    Copy = mybir.ActivationFunctionType.Copy
    ADD = mybir.AluOpType.add
    MULT = mybir.AluOpType.mult

    with tile.TileContext(nc) as tc, ExitStack() as ctx:
        cpool = ctx.enter_context(tc.tile_pool(name="const", bufs=1))
        xpool = ctx.enter_context(tc.tile_pool(name="x", bufs=3))
        scrpool = ctx.enter_context(tc.tile_pool(name="scr", bufs=1))
        opool = ctx.enter_context(tc.tile_pool(name="o", bufs=4))
        spool = ctx.enter_context(tc.tile_pool(name="small", bufs=2))
        bdpool = ctx.enter_context(tc.tile_pool(name="bd", bufs=2))
        mixp = ctx.enter_context(tc.tile_pool(name="mixp", bufs=5, space="PSUM"))
        psmall = ctx.enter_context(tc.tile_pool(name="psmall", bufs=2, space="PSUM"))

        # consts ride the gpsimd SWDGE queue so the two HWDGE rings carry
        # nothing but the bulk x/out streams
        ac_t = cpool.tile([P, N2], fp32)
        nc.gpsimd.dma_start(ac_t[:], ac)
        aux_t = cpool.tile([P, 336], fp32)
        nc.gpsimd.dma_start(aux_t[:], aux)
        sel_t = aux_t[:, 0:8]
        wcT_t = aux_t[:T, 8:72]
        id8_t = aux_t[:C, 72:80]
        rep_t = aux_t[:C, 80:208]
        mask_t = aux_t[:, 208:336]

        def phase_a(b):
            """DMA-in (split across both rings), alpha-weighted reduction, and
            the tiny k/scores/softmax chain through bd.  The chain lives here
            (not in phase_b) so every scr reader is emitted before the next
            batch's multiply reuses the single scr buffer, and so the chain's
            latency hides under the previous batch's mix."""
            # X carries dtype float32r so the BIR verifier accepts it as a
            # direct fp32r-matmul operand (f32r is bit-identical fp32; the PE
            # truncates mantissas internally). DVE reads bitcast back to fp32.
            X = xpool.tile([P, F], f32r, tag="X")
            src = xs[b].rearrange("d (j n2) t -> j d (n2 t)", j=J).bitcast(f32r)
            nc.sync.dma_start(X[: P // 2], src[: J // 2])
            nc.scalar.dma_start(X[P // 2 :], src[J // 2 :])
            # alpha-weighted product into a dedicated scratch, then a
            # contiguous in-place tree reduction over n2
            scr = scrpool.tile([P, F], fp32, tag="scr")
            nc.vector.tensor_tensor(
                out=scr[:].rearrange("p (n2 t) -> p n2 t", t=T),
                in0=X[:].bitcast(fp32).rearrange("p (n2 t) -> p n2 t", t=T),
                in1=ac_t[:].rearrange("p (x n2) -> p n2 x", x=1).to_broadcast(
                    [P, N2, T]
                ),
                op=MULT,
            )
            w = F // 2
            while w >= T:
                nc.vector.tensor_tensor(
                    out=scr[:, :w], in0=scr[:, :w], in1=scr[:, w : 2 * w], op=ADD
                )
                w //= 2

            # kT[t, d] = sum_j s[(j,d), t]  (s lives in scr[:, :T] after the tree)
            kT_ps = psmall.tile([T, C], fp32, tag="ps")
            nc.tensor.matmul(
                kT_ps[:], lhsT=scr[:, :T], rhs=sel_t, start=True, stop=True
            )
            kT_sb = spool.tile([T, C], fp32, tag="kTsb")
            nc.scalar.copy(kT_sb[:], kT_ps[:])

            # V[t, d] = sum_s Wc[t, s] k[d, s]
            v_ps = psmall.tile([T, C], fp32, tag="ps")
            nc.tensor.matmul(v_ps[:], lhsT=wcT_t, rhs=kT_sb[:], start=True, stop=True)
            v_sb = spool.tile([T, C], fp32, tag="vsb")
            nc.scalar.copy(v_sb[:], v_ps[:])

            # scores[c, d] = sum_t k[c, t] V[t, d]
            sc_ps = psmall.tile([C, C], fp32, tag="ps")
            nc.tensor.matmul(sc_ps[:], lhsT=kT_sb[:], rhs=v_sb[:], start=True, stop=True)

            # unnormalized softmax: e = exp(scores), ssum = row sums
            # (scores for this problem are bounded ~|100|: exp stays in fp32
            # range; normalization happens at PSUM evacuation)
            e_sb = spool.tile([C, C], fp32, tag="esb")
            ssum = spool.tile([C, 1], fp32, tag="ssum")
            nc.scalar.activation(e_sb[:], sc_ps[:], Exp, accum_out=ssum[:])
            rcp = spool.tile([C, 1], fp32, tag="rcp")
            nc.vector.reciprocal(rcp[:], ssum[:])

            # replicate 1/sum to mix-output partitions: rsum[(j,c), 1]
            rs_ps = psmall.tile([P, 1], fp32, tag="ps")
            nc.tensor.matmul(rs_ps[:], lhsT=rep_t, rhs=rcp[:], start=True, stop=True)
            rs_sb = spool.tile([P, 1], fp32, tag="rssb")
            nc.scalar.copy(rs_sb[:], rs_ps[:])

            # replicate e^T to all j-blocks: erep[(j,d), c] = e[c, d]
            eT_ps = psmall.tile([C, C], fp32, tag="ps")
            nc.tensor.transpose(eT_ps[:], e_sb[:], id8_t)
            eT_sb = spool.tile([C, C], fp32, tag="eTsb")
            nc.scalar.copy(eT_sb[:], eT_ps[:])
            er_ps = psmall.tile([P, C], fp32, tag="ps")
            nc.tensor.matmul(
                er_ps[:], lhsT=rep_t, rhs=eT_sb[:], start=True, stop=True
            )
            # bd[(j,d), (j',c)] = mask * erep  (block-diagonal e^T), typed
            # f32r so it can feed the fp32r mix matmuls directly
            bd = bdpool.tile([P, P], f32r, tag="bd")
            nc.vector.tensor_tensor(
                out=bd[:].rearrange("p (j c) -> p j c", j=J),
                in0=mask_t.rearrange("p (j c) -> p j c", j=J),
                in1=er_ps[:].rearrange("p (x c) -> p x c", x=1).to_broadcast([P, J, C]),
                op=MULT,
            )
            return X, bd, rs_sb

        def phase_b(b, X, bd, rs_sb):
            # channel mix (fp32r, full PE rate) + normalized bf16 evacuation,
            # half-granular staging; output halves alternate HWDGE rings
            out_b = out[b].rearrange("c (j n2) t -> j c (n2 t)", j=J)
            for h in range(2):
                ost = opool.tile([P, HF], bf16, tag="ost")
                for qq in range(HF // QW):
                    q = h * (HF // QW) + qq
                    mp = mixp.tile([P, QW], fp32, tag="mix")
                    nc.tensor.matmul(
                        mp[:],
                        lhsT=bd[:],
                        rhs=X[:, q * QW : (q + 1) * QW],
                        start=True, stop=True,
                    )
                    nc.scalar.activation(
                        ost[:, qq * QW : (qq + 1) * QW], mp[:], Copy, scale=rs_sb[:]
                    )
                dst = out_b[:, :, h * HF : (h + 1) * HF]
                if h == 0:
                    nc.scalar.dma_start(dst, ost[:])
                else:
                    nc.sync.dma_start(dst, ost[:])

        # software-pipelined emission, one batch of lookahead: per-ring FIFO
        # order becomes in(b+1), out(b), in(b+2), out(b+1), ... so input
        # prefetch is never head-of-line blocked behind an output DMA whose
        # data isn't computed yet. Lookahead stays at 1 because scr has a
        # single buffer: mul(b+1) must wait for kT(b), which phase_a(b)
        # already emitted, keeping the WAR dependency visible to the pool.
        st_prev = phase_a(0)
        for b in range(1, BS):
            st_next = phase_a(b)
            phase_b(b - 1, *st_prev)
            st_prev = st_next
        phase_b(BS - 1, *st_prev)

    nc.compile()
    return nc


def _host_constants(Wc: np.ndarray, alpha: np.ndarray):
    # ac[(j*8+d), n2] = alpha[j*128+n2]  (independent of d)
    a = alpha.reshape(J, N2).astype(np.float32)          # [16, 128]
    ac = np.repeat(a, C, axis=0)                         # [128, 128]
    # sel[(j*8+d), d'] = 1 if d == d'
    sel = np.tile(np.eye(C, dtype=np.float32), (J, 1))
    id8 = np.eye(C, dtype=np.float32)
    # rep[c', j*8+c] = 1 if c == c'  (partition replication)
    rep = np.tile(np.eye(C, dtype=np.float32), (1, J))
    # mask[(j,d), (j',c)] = 1 if j == j'
    mask = np.kron(np.eye(J, dtype=np.float32), np.ones((C, C), dtype=np.float32))
    aux = np.zeros((P, 336), dtype=np.float32)
    aux[:, 0:8] = sel
    aux[:T, 8:72] = np.asarray(Wc.T, dtype=np.float32)
    aux[:C, 72:80] = id8
    aux[:C, 80:208] = rep
    aux[:, 208:336] = mask
    return {
        "ac": np.ascontiguousarray(ac),
        "aux": aux,
    }


def get_program():
    if "nc" not in _PROGRAM_CACHE:
        _PROGRAM_CACHE["nc"] = _build_program()
    return _PROGRAM_CACHE["nc"]


def run(x, Wc, alpha, trace=False, trace_kwargs=None):
    """Run on 8 cores; returns (full_output, BassKernelResults)."""
    from concourse.bass_utils import run_bass_kernel_spmd

    nc = get_program()
    consts = _host_constants(np.asarray(Wc), np.asarray(alpha))
    x = np.asarray(x, dtype=np.float32)
    in_maps = []
    for r in range(NCORES):
        m = {"xs": np.ascontiguousarray(x[r * BS : (r + 1) * BS])}
        m.update(consts)
        in_maps.append(m)
    kw = {}
    if trace:
        kw["trace"] = True
        if trace_kwargs:
            kw.update(trace_kwargs)
    res = run_bass_kernel_spmd(nc, in_maps, list(range(NCORES)), **kw)
    out = np.concatenate(
        [np.asarray(res.results[r]["out"]).astype(np.float32) for r in range(NCORES)],
        axis=0,
    )
    return out, res


def kernel(x, Wc, alpha):
    out, _ = run(x, Wc, alpha)
    return out.astype(np.float32)


# revision 11
# speedup vs baseline: 1.1783x; 1.1783x over previous
"""Trainium2 Bass kernel for CAttention:
    k      = einsum('bcit,i->bct', x, alpha)
    scores = einsum('bct,ts,bds->bcd', k, Wc, k)
    att    = softmax(scores, axis=-1)
    out    = einsum('bci,bint->bcnt', att, x)

Sharding: data-parallel over batch B=64 across 8 NeuronCores (8 batches/core).

Per-core layout (per batch b):
    X SBUF tile [128, 8192]: partition p = j*8 + d  (j in [0,16) = n-chunk,
    d in [0,8) = channel), free q = n2*64 + t with n = j*128 + n2.

    k-path : s[(j,d),t] = sum_n2 alpha[j*128+n2] * X  (DVE mul + strided reduce)
             kT[t,d]    = sum_(j,d') s * sel          (PE, s_t as stationary)
    scores : V = Wc @ kT (PE, WcT const); scores = kT.T @ V (PE)
    softmax: unnormalized exp on ACT (accum row-sum); 1/sum replicated via PE;
             normalization folded into the PSUM-evacuation scale.
    mix    : block-diag(e^T) [128,128] stationary, fp32r matmuls (1 cyc/row)
    out    : ACT evacuates PSUM -> SBUF bf16 with per-partition 1/sum scale,
             DMA out in bf16 (host upcasts); halves the write traffic.

DMA ring balance: each batch's input is split into two partition-halves, one
on the SP (sync) HWDGE ring and one on the ACT (scalar) ring; each batch's
output (bf16, staged in halves) likewise alternates rings. Both rings carry
~3.2MB/batch so the 16 SDMA engines always have two queues to round-robin
(measured: one loaded ring sustains ~210 GB/s, two loaded rings ~300+).
Emission is software-pipelined one batch ahead (a0 a1 b0 a2 b1 ... b7) so
input prefetch descriptors are never queued behind output DMAs on a ring.
Constants ride the gpsimd SWDGE queue to keep the HWDGE rings clean.
"""

import sys

for _p in ("/opt/trn_rl_repo",):
    if _p not in sys.path:
        sys.path.insert(0, _p)

import numpy as np

B, C, N, T = 64, 8, 2048, 64
NCORES = 8
BS = B // NCORES          # batches per core
J = 16                    # n-chunks on partitions
N2 = N // J               # 128, n-extent in free dim
P = J * C                 # 128 partitions
F = N2 * T                # 8192 free elems
QW = 512                  # mix matmul free width (one PSUM bank)
HF = F // 2               # output staging half

_PROGRAM_CACHE = {}


def _build_program():
    from contextlib import ExitStack

    import concourse.bacc as bacc
    from concourse import mybir, tile

    fp32 = mybir.dt.float32
    f32r = mybir.dt.float32r
    bf16 = mybir.dt.bfloat16
    nc = bacc.Bacc("TRN2", target_bir_lowering=False, debug=False)

    xs = nc.dram_tensor("xs", [BS, C, N, T], fp32, kind="ExternalInput").ap()
    ac = nc.dram_tensor("ac", [P, N2], fp32, kind="ExternalInput").ap()
    # packed: sel[0:8] | wcT[8:72] (rows 0-63) | id8[72:80] (rows 0-7) |
    #         rep[80:208] (rows 0-7) | mask[208:336]
    aux = nc.dram_tensor("aux", [P, 336], fp32, kind="ExternalInput").ap()
    out = nc.dram_tensor("out", [BS, C, N, T], bf16, kind="ExternalOutput").ap()

    Exp = mybir.ActivationFunctionType.Exp
    Copy = mybir.ActivationFunctionType.Copy
    ADD = mybir.AluOpType.add
    MULT = mybir.AluOpType.mult

    with tile.TileContext(nc) as tc, ExitStack() as ctx:
        cpool = ctx.enter_context(tc.tile_pool(name="const", bufs=1))
        xpool = ctx.enter_context(tc.tile_pool(name="x", bufs=3))
        scrpool = ctx.enter_context(tc.tile_pool(name="scr", bufs=1))
        opool = ctx.enter_context(tc.tile_pool(name="o", bufs=4))
        spool = ctx.enter_context(tc.tile_pool(name="small", bufs=2))
        bdpool = ctx.enter_context(tc.tile_pool(name="bd", bufs=2))
        mixp = ctx.enter_context(tc.tile_pool(name="mixp", bufs=5, space="PSUM"))
        psmall = ctx.enter_context(tc.tile_pool(name="psmall", bufs=2, space="PSUM"))

        # consts ride the gpsimd SWDGE queue so the two HWDGE rings carry
        # nothing but the bulk x/out streams
        ac_t = cpool.tile([P, N2], fp32)
        nc.gpsimd.dma_start(ac_t[:], ac)
        aux_t = cpool.tile([P, 336], fp32)
        nc.gpsimd.dma_start(aux_t[:], aux)
        sel_t = aux_t[:, 0:8]
        wcT_t = aux_t[:T, 8:72]
        id8_t = aux_t[:C, 72:80]
        rep_t = aux_t[:C, 80:208]
        mask_t = aux_t[:, 208:336]

        def phase_a(b):
            """DMA-in (split across both rings), alpha-weighted reduction, and
            the tiny k/scores/softmax chain through bd.  The chain lives here
            (not in phase_b) so every scr reader is emitted before the next
            batch's multiply reuses the single scr buffer, and so the chain's
            latency hides under the previous batch's mix."""
            # X carries dtype float32r so the BIR verifier accepts it as a
            # direct fp32r-matmul operand (f32r is bit-identical fp32; the PE
            # truncates mantissas internally). DVE reads bitcast back to fp32.
            X = xpool.tile([P, F], f32r, tag="X")
            src = xs[b].rearrange("d (j n2) t -> j d (n2 t)", j=J).bitcast(f32r)
            nc.sync.dma_start(X[: P // 2], src[: J // 2])
            nc.scalar.dma_start(X[P // 2 :], src[J // 2 :])
            # alpha-weighted product into a dedicated scratch, then a
            # contiguous in-place tree reduction over n2
            scr = scrpool.tile([P, F], fp32, tag="scr")
            nc.vector.tensor_tensor(
                out=scr[:].rearrange("p (n2 t) -> p n2 t", t=T),
                in0=X[:].bitcast(fp32).rearrange("p (n2 t) -> p n2 t", t=T),
                in1=ac_t[:].rearrange("p (x n2) -> p n2 x", x=1).to_broadcast(
                    [P, N2, T]
                ),
                op=MULT,
            )
            w = F // 2
            while w >= T:
                nc.vector.tensor_tensor(
                    out=scr[:, :w], in0=scr[:, :w], in1=scr[:, w : 2 * w], op=ADD
                )
                w //= 2

            # kT[t, d] = sum_j s[(j,d), t]  (s lives in scr[:, :T] after the tree)
            kT_ps = psmall.tile([T, C], fp32, tag="ps")
            nc.tensor.matmul(
                kT_ps[:], lhsT=scr[:, :T], rhs=sel_t, start=True, stop=True
            )
            kT_sb = spool.tile([T, C], fp32, tag="kTsb")
            nc.scalar.copy(kT_sb[:], kT_ps[:])

            # V[t, d] = sum_s Wc[t, s] k[d, s]
            v_ps = psmall.tile([T, C], fp32, tag="ps")
            nc.tensor.matmul(v_ps[:], lhsT=wcT_t, rhs=kT_sb[:], start=True, stop=True)
            v_sb = spool.tile([T, C], fp32, tag="vsb")
            nc.scalar.copy(v_sb[:], v_ps[:])

            # scores[c, d] = sum_t k[c, t] V[t, d]
            sc_ps = psmall.tile([C, C], fp32, tag="ps")
            nc.tensor.matmul(sc_ps[:], lhsT=kT_sb[:], rhs=v_sb[:], start=True, stop=True)

            # unnormalized softmax: e = exp(scores), ssum = row sums
            # (scores for this problem are bounded ~|100|: exp stays in fp32
            # range; normalization happens at PSUM evacuation)
            e_sb = spool.tile([C, C], fp32, tag="esb")
            ssum = spool.tile([C, 1], fp32, tag="ssum")
            nc.scalar.activation(e_sb[:], sc_ps[:], Exp, accum_out=ssum[:])
            rcp = spool.tile([C, 1], fp32, tag="rcp")
            nc.vector.reciprocal(rcp[:], ssum[:])

            # replicate 1/sum to mix-output partitions: rsum[(j,c), 1]
            rs_ps = psmall.tile([P, 1], fp32, tag="ps")
            nc.tensor.matmul(rs_ps[:], lhsT=rep_t, rhs=rcp[:], start=True, stop=True)
            rs_sb = spool.tile([P, 1], fp32, tag="rssb")
            nc.scalar.copy(rs_sb[:], rs_ps[:])

            # replicate e^T to all j-blocks: erep[(j,d), c] = e[c, d]
            eT_ps = psmall.tile([C, C], fp32, tag="ps")
            nc.tensor.transpose(eT_ps[:], e_sb[:], id8_t)
            eT_sb = spool.tile([C, C], fp32, tag="eTsb")
            nc.scalar.copy(eT_sb[:], eT_ps[:])
            er_ps = psmall.tile([P, C], fp32, tag="ps")
            nc.tensor.matmul(
                er_ps[:], lhsT=rep_t, rhs=eT_sb[:], start=True, stop=True
            )
            # bd[(j,d), (j',c)] = mask * erep  (block-diagonal e^T), typed
            # f32r so it can feed the fp32r mix matmuls directly
            bd = bdpool.tile([P, P], f32r, tag="bd")
            nc.vector.tensor_tensor(
                out=bd[:].rearrange("p (j c) -> p j c", j=J),
                in0=mask_t.rearrange("p (j c) -> p j c", j=J),
                in1=er_ps[:].rearrange("p (x c) -> p x c", x=1).to_broadcast([P, J, C]),
                op=MULT,
            )
            return X, bd, rs_sb

        def phase_b(b, X, bd, rs_sb):
            # channel mix (fp32r, full PE rate) + normalized bf16 evacuation,
            # half-granular staging; output halves alternate HWDGE rings
            out_b = out[b].rearrange("c (j n2) t -> j c (n2 t)", j=J)
            for h in range(2):
                ost = opool.tile([P, HF], bf16, tag="ost")
                for qq in range(HF // QW):
                    q = h * (HF // QW) + qq
                    mp = mixp.tile([P, QW], fp32, tag="mix")
                    nc.tensor.matmul(
                        mp[:],
                        lhsT=bd[:],
                        rhs=X[:, q * QW : (q + 1) * QW],
                        start=True, stop=True,
                    )
                    nc.scalar.activation(
                        ost[:, qq * QW : (qq + 1) * QW], mp[:], Copy, scale=rs_sb[:]
                    )
                # outputs ride the gpsimd SWDGE queue: a third DMA stream the
                # SDMA engines round-robin with the two input rings, and one
                # whose issue (gated on evac completion) can never head-of-line
                # block input prefetch descriptors on the HWDGE rings
                nc.gpsimd.dma_start(out_b[:, :, h * HF : (h + 1) * HF], ost[:])

        for b in range(BS):
            phase_b(b, *phase_a(b))

    nc.compile()
    return nc


def _host_constants(Wc: np.ndarray, alpha: np.ndarray):
    # ac[(j*8+d), n2] = alpha[j*128+n2]  (independent of d)
    a = alpha.reshape(J, N2).astype(np.float32)          # [16, 128]
    ac = np.repeat(a, C, axis=0)                         # [128, 128]
    # sel[(j*8+d), d'] = 1 if d == d'
    sel = np.tile(np.eye(C, dtype=np.float32), (J, 1))
    id8 = np.eye(C, dtype=np.float32)
    # rep[c', j*8+c] = 1 if c == c'  (partition replication)
    rep = np.tile(np.eye(C, dtype=np.float32), (1, J))
    # mask[(j,d), (j',c)] = 1 if j == j'
    mask = np.kron(np.eye(J, dtype=np.float32), np.ones((C, C), dtype=np.float32))
    aux = np.zeros((P, 336), dtype=np.float32)
    aux[:, 0:8] = sel
    aux[:T, 8:72] = np.asarray(Wc.T, dtype=np.float32)
    aux[:C, 72:80] = id8
    aux[:C, 80:208] = rep
    aux[:, 208:336] = mask
    return {
        "ac": np.ascontiguousarray(ac),
        "aux": aux,
    }


def get_program():
    if "nc" not in _PROGRAM_CACHE:
        _PROGRAM_CACHE["nc"] = _build_program()
    return _PROGRAM_CACHE["nc"]


def run(x, Wc, alpha, trace=False, trace_kwargs=None):
    """Run on 8 cores; returns (full_output, BassKernelResults)."""
    from concourse.bass_utils import run_bass_kernel_spmd

    nc = get_program()
    consts = _host_constants(np.asarray(Wc), np.asarray(alpha))
    x = np.asarray(x, dtype=np.float32)
    in_maps = []
    for r in range(NCORES):
        m = {"xs": np.ascontiguousarray(x[r * BS : (r + 1) * BS])}
        m.update(consts)
        in_maps.append(m)
    kw = {}
    if trace:
        kw["trace"] = True
        if trace_kwargs:
            kw.update(trace_kwargs)
    res = run_bass_kernel_spmd(nc, in_maps, list(range(NCORES)), **kw)
    out = np.concatenate(
        [np.asarray(res.results[r]["out"]).astype(np.float32) for r in range(NCORES)],
        axis=0,
    )
    return out, res


def kernel(x, Wc, alpha):
    out, _ = run(x, Wc, alpha)
    return out.astype(np.float32)


# revision 13
# speedup vs baseline: 1.5231x; 1.2926x over previous
"""Trainium2 Bass kernel for CAttention:
    k      = einsum('bcit,i->bct', x, alpha)
    scores = einsum('bct,ts,bds->bcd', k, Wc, k)
    att    = softmax(scores, axis=-1)
    out    = einsum('bci,bint->bcnt', att, x)

Sharding: data-parallel over batch B=64 across 8 NeuronCores (8 batches/core).

Per-core layout (per batch b):
    X SBUF tile [128, 8192]: partition p = j*8 + d  (j in [0,16) = n-chunk,
    d in [0,8) = channel), free q = n2*64 + t with n = j*128 + n2.

    k-path : s[(j,d),t] = sum_n2 alpha[j*128+n2] * X  (DVE mul + strided reduce)
             kT[t,d]    = sum_(j,d') s * sel          (PE, s_t as stationary)
    scores : V = Wc @ kT (PE, WcT const); scores = kT.T @ V (PE)
    softmax: unnormalized exp on ACT (accum row-sum); 1/sum replicated via PE;
             normalization folded into the PSUM-evacuation scale.
    mix    : block-diag(e^T) [128,128] stationary, fp32r matmuls (1 cyc/row)
    out    : ACT evacuates PSUM -> SBUF bf16 with per-partition 1/sum scale,
             DMA out in bf16 (host upcasts); halves the write traffic.

DMA ring balance: each batch's input is split into two partition-halves, one
on the SP (sync) HWDGE ring and one on the ACT (scalar) ring; each batch's
output (bf16, staged in halves) likewise alternates rings. Both rings carry
~3.2MB/batch so the 16 SDMA engines always have two queues to round-robin
(measured: one loaded ring sustains ~210 GB/s, two loaded rings ~300+).
Emission is software-pipelined one batch ahead (a0 a1 b0 a2 b1 ... b7) so
input prefetch descriptors are never queued behind output DMAs on a ring.
Constants ride the gpsimd SWDGE queue to keep the HWDGE rings clean.
"""

import sys

for _p in ("/opt/trn_rl_repo",):
    if _p not in sys.path:
        sys.path.insert(0, _p)

import numpy as np

B, C, N, T = 64, 8, 2048, 64
NCORES = 8
BS = B // NCORES          # batches per core
J = 16                    # n-chunks on partitions
N2 = N // J               # 128, n-extent in free dim
P = J * C                 # 128 partitions
F = N2 * T                # 8192 free elems
QW = 512                  # mix matmul free width (one PSUM bank)
HF = F // 2               # output staging half

_PROGRAM_CACHE = {}


def _build_program():
    from contextlib import ExitStack

    import concourse.bacc as bacc
    from concourse import mybir, tile

    fp32 = mybir.dt.float32
    f32r = mybir.dt.float32r
    bf16 = mybir.dt.bfloat16
    nc = bacc.Bacc("TRN2", target_bir_lowering=False, debug=False)

    xs = nc.dram_tensor("xs", [BS, C, N, T], fp32, kind="ExternalInput").ap()
    ac = nc.dram_tensor("ac", [P, N2], fp32, kind="ExternalInput").ap()
    # packed: sel[0:8] | wcT[8:72] (rows 0-63) | id8[72:80] (rows 0-7) |
    #         rep[80:208] (rows 0-7) | mask[208:336]
    aux = nc.dram_tensor("aux", [P, 336], fp32, kind="ExternalInput").ap()
    out = nc.dram_tensor("out", [BS, C, N, T], bf16, kind="ExternalOutput").ap()

    Exp = mybir.ActivationFunctionType.Exp
    Copy = mybir.ActivationFunctionType.Copy
    ADD = mybir.AluOpType.add
    MULT = mybir.AluOpType.mult

    with tile.TileContext(nc) as tc, ExitStack() as ctx:
        cpool = ctx.enter_context(tc.tile_pool(name="const", bufs=1))
        xpool = ctx.enter_context(tc.tile_pool(name="x", bufs=3))
        scrpool = ctx.enter_context(tc.tile_pool(name="scr", bufs=1))
        opool = ctx.enter_context(tc.tile_pool(name="o", bufs=4))
        spool = ctx.enter_context(tc.tile_pool(name="small", bufs=2))
        bdpool = ctx.enter_context(tc.tile_pool(name="bd", bufs=2))
        mixp = ctx.enter_context(tc.tile_pool(name="mixp", bufs=5, space="PSUM"))
        psmall = ctx.enter_context(tc.tile_pool(name="psmall", bufs=2, space="PSUM"))

        # consts ride the gpsimd SWDGE queue so the two HWDGE rings carry
        # nothing but the bulk x/out streams
        ac_t = cpool.tile([P, N2], fp32)
        nc.gpsimd.dma_start(ac_t[:], ac)
        aux_t = cpool.tile([P, 336], fp32)
        nc.gpsimd.dma_start(aux_t[:], aux)
        sel_t = aux_t[:, 0:8]
        wcT_t = aux_t[:T, 8:72]
        id8_t = aux_t[:C, 72:80]
        rep_t = aux_t[:C, 80:208]
        mask_t = aux_t[:, 208:336]

        def phase_a(b):
            """DMA-in (split across both rings), alpha-weighted reduction, and
            the tiny k/scores/softmax chain through bd.  The chain lives here
            (not in phase_b) so every scr reader is emitted before the next
            batch's multiply reuses the single scr buffer, and so the chain's
            latency hides under the previous batch's mix."""
            # X carries dtype float32r so the BIR verifier accepts it as a
            # direct fp32r-matmul operand (f32r is bit-identical fp32; the PE
            # truncates mantissas internally). DVE reads bitcast back to fp32.
            # input halves split along the FREE dim (n2), one per HWDGE ring:
            # every DMA spans all 128 partitions (descriptors map to SDMA
            # engines by partition, so a partition-split would engage only
            # half the engines), and each partition line stays a contiguous
            # 16KB HBM read
            X = xpool.tile([P, F], f32r, tag="X")
            src = xs[b].rearrange("d (j n2) t -> j d (n2 t)", j=J).bitcast(f32r)
            nc.sync.dma_start(X[:, : F // 2], src[:, :, : F // 2])
            nc.scalar.dma_start(X[:, F // 2 :], src[:, :, F // 2 :])
            # alpha-weighted product into a dedicated scratch, then a
            # contiguous in-place tree reduction over n2
            scr = scrpool.tile([P, F], fp32, tag="scr")
            nc.vector.tensor_tensor(
                out=scr[:].rearrange("p (n2 t) -> p n2 t", t=T),
                in0=X[:].bitcast(fp32).rearrange("p (n2 t) -> p n2 t", t=T),
                in1=ac_t[:].rearrange("p (x n2) -> p n2 x", x=1).to_broadcast(
                    [P, N2, T]
                ),
                op=MULT,
            )
            w = F // 2
            while w >= T:
                nc.vector.tensor_tensor(
                    out=scr[:, :w], in0=scr[:, :w], in1=scr[:, w : 2 * w], op=ADD
                )
                w //= 2

            # kT[t, d] = sum_j s[(j,d), t]  (s lives in scr[:, :T] after the tree)
            kT_ps = psmall.tile([T, C], fp32, tag="ps")
            nc.tensor.matmul(
                kT_ps[:], lhsT=scr[:, :T], rhs=sel_t, start=True, stop=True
            )
            kT_sb = spool.tile([T, C], fp32, tag="kTsb")
            nc.scalar.copy(kT_sb[:], kT_ps[:])

            # V[t, d] = sum_s Wc[t, s] k[d, s]
            v_ps = psmall.tile([T, C], fp32, tag="ps")
            nc.tensor.matmul(v_ps[:], lhsT=wcT_t, rhs=kT_sb[:], start=True, stop=True)
            v_sb = spool.tile([T, C], fp32, tag="vsb")
            nc.scalar.copy(v_sb[:], v_ps[:])

            # scores[c, d] = sum_t k[c, t] V[t, d]
            sc_ps = psmall.tile([C, C], fp32, tag="ps")
            nc.tensor.matmul(sc_ps[:], lhsT=kT_sb[:], rhs=v_sb[:], start=True, stop=True)

            # unnormalized softmax: e = exp(scores), ssum = row sums
            # (scores for this problem are bounded ~|100|: exp stays in fp32
            # range; normalization happens at PSUM evacuation)
            e_sb = spool.tile([C, C], fp32, tag="esb")
            ssum = spool.tile([C, 1], fp32, tag="ssum")
            nc.scalar.activation(e_sb[:], sc_ps[:], Exp, accum_out=ssum[:])
            rcp = spool.tile([C, 1], fp32, tag="rcp")
            nc.vector.reciprocal(rcp[:], ssum[:])

            # replicate 1/sum to mix-output partitions: rsum[(j,c), 1]
            rs_ps = psmall.tile([P, 1], fp32, tag="ps")
            nc.tensor.matmul(rs_ps[:], lhsT=rep_t, rhs=rcp[:], start=True, stop=True)
            rs_sb = spool.tile([P, 1], fp32, tag="rssb")
            nc.scalar.copy(rs_sb[:], rs_ps[:])

            # replicate e^T to all j-blocks: erep[(j,d), c] = e[c, d]
            eT_ps = psmall.tile([C, C], fp32, tag="ps")
            nc.tensor.transpose(eT_ps[:], e_sb[:], id8_t)
            eT_sb = spool.tile([C, C], fp32, tag="eTsb")
            nc.scalar.copy(eT_sb[:], eT_ps[:])
            er_ps = psmall.tile([P, C], fp32, tag="ps")
            nc.tensor.matmul(
                er_ps[:], lhsT=rep_t, rhs=eT_sb[:], start=True, stop=True
            )
            # bd[(j,d), (j',c)] = mask * erep  (block-diagonal e^T), typed
            # f32r so it can feed the fp32r mix matmuls directly
            bd = bdpool.tile([P, P], f32r, tag="bd")
            nc.vector.tensor_tensor(
                out=bd[:].rearrange("p (j c) -> p j c", j=J),
                in0=mask_t.rearrange("p (j c) -> p j c", j=J),
                in1=er_ps[:].rearrange("p (x c) -> p x c", x=1).to_broadcast([P, J, C]),
                op=MULT,
            )
            return X, bd, rs_sb

        def phase_b(b, X, bd, rs_sb):
            # channel mix (fp32r, full PE rate) + normalized bf16 evacuation,
            # half-granular staging; output halves alternate HWDGE rings
            out_b = out[b].rearrange("c (j n2) t -> j c (n2 t)", j=J)
            for h in range(2):
                ost = opool.tile([P, HF], bf16, tag="ost")
                for qq in range(HF // QW):
                    q = h * (HF // QW) + qq
                    mp = mixp.tile([P, QW], fp32, tag="mix")
                    nc.tensor.matmul(
                        mp[:],
                        lhsT=bd[:],
                        rhs=X[:, q * QW : (q + 1) * QW],
                        start=True, stop=True,
                    )
                    nc.scalar.activation(
                        ost[:, qq * QW : (qq + 1) * QW], mp[:], Copy, scale=rs_sb[:]
                    )
                # outputs ride the gpsimd SWDGE queue: a third DMA stream the
                # SDMA engines round-robin with the two input rings, and one
                # whose issue (gated on evac completion) can never head-of-line
                # block input prefetch descriptors on the HWDGE rings
                nc.gpsimd.dma_start(out_b[:, :, h * HF : (h + 1) * HF], ost[:])

        for b in range(BS):
            phase_b(b, *phase_a(b))

    nc.compile()
    return nc


def _host_constants(Wc: np.ndarray, alpha: np.ndarray):
    # ac[(j*8+d), n2] = alpha[j*128+n2]  (independent of d)
    a = alpha.reshape(J, N2).astype(np.float32)          # [16, 128]
    ac = np.repeat(a, C, axis=0)                         # [128, 128]
    # sel[(j*8+d), d'] = 1 if d == d'
    sel = np.tile(np.eye(C, dtype=np.float32), (J, 1))
    id8 = np.eye(C, dtype=np.float32)
    # rep[c', j*8+c] = 1 if c == c'  (partition replication)
    rep = np.tile(np.eye(C, dtype=np.float32), (1, J))
    # mask[(j,d), (j',c)] = 1 if j == j'
    mask = np.kron(np.eye(J, dtype=np.float32), np.ones((C, C), dtype=np.float32))
    aux = np.zeros((P, 336), dtype=np.float32)
    aux[:, 0:8] = sel
    aux[:T, 8:72] = np.asarray(Wc.T, dtype=np.float32)
    aux[:C, 72:80] = id8
    aux[:C, 80:208] = rep
    aux[:, 208:336] = mask
    return {
        "ac": np.ascontiguousarray(ac),
        "aux": aux,
    }


def get_program():
    if "nc" not in _PROGRAM_CACHE:
        _PROGRAM_CACHE["nc"] = _build_program()
    return _PROGRAM_CACHE["nc"]


def run(x, Wc, alpha, trace=False, trace_kwargs=None):
    """Run on 8 cores; returns (full_output, BassKernelResults)."""
    from concourse.bass_utils import run_bass_kernel_spmd

    nc = get_program()
    consts = _host_constants(np.asarray(Wc), np.asarray(alpha))
    x = np.asarray(x, dtype=np.float32)
    in_maps = []
    for r in range(NCORES):
        m = {"xs": np.ascontiguousarray(x[r * BS : (r + 1) * BS])}
        m.update(consts)
        in_maps.append(m)
    kw = {}
    if trace:
        kw["trace"] = True
        if trace_kwargs:
            kw.update(trace_kwargs)
    res = run_bass_kernel_spmd(nc, in_maps, list(range(NCORES)), **kw)
    out = np.concatenate(
        [np.asarray(res.results[r]["out"]).astype(np.float32) for r in range(NCORES)],
        axis=0,
    )
    return out, res


def kernel(x, Wc, alpha):
    out, _ = run(x, Wc, alpha)
    return out.astype(np.float32)
